# revision 2
# baseline (speedup 1.0000x reference)
"""Haar DWT2 (pywt 'periodization', single level) on Trainium2, 8 NeuronCores.

Input  x: (8, 64, 512, 512) f32
Output (ll, lh, hl, hh): each (8, 64, 256, 256) f32

Math (non-overlapping 2x2 blocks):
  a=x[2i,2j], b=x[2i,2j+1], c=x[2i+1,2j], d=x[2i+1,2j+1]
  ll=(a+b+c+d)/2, lh=(a+b-c-d)/2, hl=(a-b+c-d)/2, hh=(a-b-c+d)/2

Strategy: fully data-parallel across 8 cores (batch dim), fp16 on device.
The host folds the *0.5 into the f32->f16 cast and de-interleaves even/odd
columns, staging each core's input as xi[row, colparity, 256] so every
device-side op is a dense step-1 fp16 tensor_tensor (DVE 2x perf mode):
  column pass: S = xe + xo, D = xe - xo            (both rows of each pair)
  row pass:    ll = S_even + S_odd, lh = S_even - S_odd,
               hl = D_even + D_odd, hh = D_even - D_odd
The four subbands are packed into one output tensor y[rowpair, 4, 256] so
each tile has exactly one input DMA (2 MiB, sync/HWDGE) and one output DMA
(2 MiB, scalar/HWDGE). HBM traffic: 32 MiB in + 32 MiB out per core =>
~165 us roofline at ~400 GB/s effective.
"""

import sys

if "/opt/trn_rl_repo" not in sys.path:
    sys.path.insert(0, "/opt/trn_rl_repo")

import numpy as np

N_CORES = 8
P = 128  # SBUF partitions


def _ensure_axon_ntff_hook():
    """The image's antenv package lacks the axon_hooks glue module that
    run_bass_kernel_spmd imports when tracing is requested (BASS_TRACE).
    Recreate it so traced runs work; harmless if already present."""
    try:
        import antenv.axon_hooks  # noqa: F401

        return
    except ImportError:
        pass
    try:
        import types

        import antenv
        from trn_agent_boot.trn_boot import _ntff_profile_via_ctypes

        mod = types.ModuleType("antenv.axon_hooks")
        holder = [None]
        mod.set_axon_ntff_profile_hook = lambda h: holder.__setitem__(0, h)
        mod.get_axon_ntff_profile_hook = lambda: holder[0]
        sys.modules["antenv.axon_hooks"] = mod
        antenv.axon_hooks = mod
        mod.set_axon_ntff_profile_hook(
            _ntff_profile_via_ctypes("/opt/axon/libaxon_pjrt.so")
        )
    except Exception:
        pass


def build_dwt_program(n_rows, Wh, R, debug=False, compile=True):
    """Bass program for one core.

    x [n_rows, 2, Wh] f16  (row, col-parity, col/2; rows h-major)
      -> y [n_rows//2, 4, Wh] f16  (rowpair, subband ll/lh/hl/hh, col/2)
    R = rowpairs per partition per tile.
    """
    from concourse import bacc, tile
    import concourse.mybir as mybir

    f16 = mybir.dt.float16
    add = mybir.AluOpType.add
    sub = mybir.AluOpType.subtract

    nc = bacc.Bacc("TRN2", target_bir_lowering=False, debug=debug)
    x = nc.dram_tensor("x", [n_rows, 2, Wh], f16, kind="ExternalInput")
    y = nc.dram_tensor("y", [n_rows // 2, 4, Wh], f16, kind="ExternalOutput")

    rows_per_tile = P * 2 * R
    assert n_rows % rows_per_tile == 0
    n_tiles = n_rows // rows_per_tile

    with tile.TileContext(nc) as tc:
        with tc.tile_pool(name="io", bufs=3) as pool:
            for t in range(n_tiles):
                rsl = slice(t * rows_per_tile, (t + 1) * rows_per_tile)
                psl = slice(t * rows_per_tile // 2, (t + 1) * rows_per_tile // 2)
                # One input DMA: 2R rows x 512 f16 = 2R KiB contiguous/partition.
                T = pool.tile([P, 2 * R, 2, Wh], f16, tag="T")
                nc.sync.dma_start(
                    out=T[:], in_=x[rsl].rearrange("(q j) e w -> q j e w", q=P)
                )
                # Column pass (all rows): S = xe + xo, D = xe - xo.
                S = pool.tile([P, R, 2, Wh], f16, tag="S")
                D = pool.tile([P, R, 2, Wh], f16, tag="D")
                Sv = S.rearrange("p r a w -> p (r a) w")
                Dv = D.rearrange("p r a w -> p (r a) w")
                nc.vector.tensor_tensor(Sv, T[:, :, 0, :], T[:, :, 1, :], add)
                nc.vector.tensor_tensor(Dv, T[:, :, 0, :], T[:, :, 1, :], sub)
                # Row pass into the packed output tile.
                U = pool.tile([P, R, 4, Wh], f16, tag="U")
                for k, (src, op) in enumerate(
                    ((S, add), (S, sub), (D, add), (D, sub))
                ):
                    nc.vector.tensor_tensor(
                        U[:, :, k, :], src[:, :, 0, :], src[:, :, 1, :], op
                    )
                # One output DMA: 4R x 256 f16 = 2R KiB contiguous/partition.
                nc.scalar.dma_start(
                    out=y[psl].rearrange("(q r) s w -> q r s w", q=P), in_=U[:]
                )
    if compile:
        nc.compile()
    return nc


_program_cache = {}


def _get_program(n_rows=32768, Wh=256, R=8):
    key = (n_rows, Wh, R)
    if key not in _program_cache:
        _program_cache[key] = build_dwt_program(n_rows, Wh, R)
    return _program_cache[key]


def _prep_core_input(xc):
    """[C, H, W] f32 -> [C*H, 2, W//2] f16, scaled by 0.5, cols de-interleaved."""
    C, H, W = xc.shape
    v = xc.reshape(C, H, W // 2, 2).transpose(0, 1, 3, 2)
    out = np.empty((C, H, 2, W // 2), np.float16)
    np.multiply(v, np.float32(0.5), out=out, casting="unsafe")
    return out.reshape(C * H, 2, W // 2)


def prepare_in_maps(x):
    """Full (8, C, H, W) f32 input -> per-core in_maps for the bass program."""
    from concurrent.futures import ThreadPoolExecutor

    with ThreadPoolExecutor(N_CORES) as ex:
        xs = list(ex.map(_prep_core_input, [x[c] for c in range(N_CORES)]))
    return [{"x": xc} for xc in xs]


def finalize_outputs(res, C, H, W):
    """Per-core y [C*H//2, 4, W//2] f16 -> (ll, lh, hl, hh) full f32."""
    out = tuple(
        np.empty((N_CORES, C, H // 2, W // 2), np.float32) for _ in range(4)
    )
    for c in range(N_CORES):
        yc = res[c]["y"].reshape(C, H // 2, 4, W // 2)
        for k in range(4):
            out[k][c] = yc[:, :, k, :]
    return out


def kernel(x_input):
    from concourse.bass_utils import run_bass_kernel_spmd

    _ensure_axon_ntff_hook()

    x = np.asarray(x_input)
    B, C, H, W = x.shape  # (8, 64, 512, 512)
    assert B == N_CORES
    nc = _get_program(C * H, W // 2, R=8)
    in_maps = prepare_in_maps(x)
    res = run_bass_kernel_spmd(nc, in_maps, list(range(N_CORES))).results
    return finalize_outputs(res, C, H, W)
